# revision 1
# baseline (speedup 1.0000x reference)
"""Trainium2 Bass kernel for nn_CBAE_EndToEnd — pixel-major active-prim
compaction design.

Key idea: each image row (128 pixels) only intersects ~20 of the 128
primitives (soft convex intersection of 12 random half-planes). Host
computes, per (frame, row), the conservative active-prim list (interval
test with sigmoid-saturation margin), sorts rows by count into slots,
groups slots adaptively with data-derived uniform capacity, and packs
edge coefficients into a pitch-14 fp16 matmul stream per prim:
  [x-col: logit(aeff) | e0..e11 edge cols | R reset col]
so that sigma(x-col) = aeff exactly folds opacity into the edge product.

Device per slot (pixels of the row on partitions, prims/edges on free):
  PE    : arg = A*gx + C' via contract-4 fp16 matmul (gx is exact fp16,
          A/C' 2-split), static lhsT [gx, gx, 1, 1].
  ACT   : sigmoid over 2-PSUM-bank batches (only table used).
  DVE   : per-slot product scan  state = max(state*sigma, d1)  with
          d1=1 at R cols -> readout a_i = aeff*prod sigma at e11 cols;
          one_m = 1-a; per-group compositing scan (cumprod of 1-a with
          per-slot reset); w_i = t_excl - t_incl = a_i * t_excl via
          shifted subtract.  fp16 everywhere (scan state is fp32
          internally), 2x/4x DVE perf modes.
  PE    : batched fp16 transposes of w (2 slots/chunk at 32/64-part
          pitch), 3-col color matmuls per slot into one shared PSUM
          bank per frame.
  Pool  : PSUM->SBUF copies of transposed w.
Output [frame, slot, pix, 3]; host un-permutes slots back to rows.
"""

import numpy as np

H = 128
W = 128
N = 128
K = 12
SOFT = 0.01
T_TOTAL = 192
N_CORES = 8
F = T_TOTAL // N_CORES
MARGIN = 9.0           # |arg| beyond this counts as saturated
PITCH = K + 2          # x-col + 12 edges + reset col
NSLOT = 128            # one slot per image row
BANK = 512             # fp32 cols per PSUM bank
GLIM = 4096            # max matmul-stream cols per group (8 banks)

fp16 = np.float16

_CACHE = {}


# ---------------------------------------------------------------------------
# host prep
# ---------------------------------------------------------------------------

def _split2(x):
    x = np.asarray(x, np.float32)
    h = x.astype(fp16)
    l = (x - h.astype(np.float32)).astype(fp16)
    return h, l


def _make_groups(capr):
    """Greedy grouping of sorted slots: uniform cap per group, bounded
    column footprint.  capr[r] = max over frames of r-th smallest count."""
    groups = []   # list of dicts
    s = 0
    col_off = 0
    ck_off = 0
    while s < NSLOT:
        n = 1
        while s + n < NSLOT and n < 16:
            cap = max(1, int(capr[s + n]))
            if PITCH * cap * (n + 1) > GLIM:
                break
            n += 1
        C = max(1, int(capr[s + n - 1]))
        cols = PITCH * C * n
        nb = (cols + BANK - 1) // BANK
        cpitch = 64 if C <= 64 else 128
        spc = 2 if C <= 64 else 1
        groups.append(dict(s0=s, n=n, C=C, off=col_off, nb=nb,
                           cpitch=cpitch, spc=spc, ckoff=ck_off))
        col_off += nb * BANK
        ck_off += 3 * n
        s += n
    return groups, col_off // BANK, ck_off


def _groups_key(groups):
    return tuple((g["s0"], g["n"], g["C"]) for g in groups)


def _plan(trajectory, alpha, z, csg, colors):
    """Compute compaction plan + packed per-frame data for ALL frames."""
    T = trajectory.shape[0]
    od = np.argsort(z, kind="stable")[::-1]     # descending z = paint order
    traj = np.asarray(trajectory, np.float32)[:, 0, :]
    P = traj[:, : N * K * 2].reshape(T, N, K, 2)[:, od]
    alive = traj[:, N * K * 2:][:, od]
    v0 = P
    v1 = np.roll(P, -1, axis=2)
    e = v1 - v0
    area2 = np.sum(v0[..., 0] * v1[..., 1] - v1[..., 0] * v0[..., 1], axis=2)
    orient = np.sign(area2).astype(np.float32)[:, :, None]
    A = (-orient * e[..., 1] / SOFT).astype(np.float32)       # [T,N,K] gx coef
    B = (orient * e[..., 0] / SOFT).astype(np.float32)        # gy coef
    Cc = (orient * (e[..., 1] * v0[..., 0] - e[..., 0] * v0[..., 1]) / SOFT
          ).astype(np.float32)

    sig_alive = 1.0 / (1.0 + np.exp(-alive.astype(np.float32)))
    aeff = np.asarray(alpha, np.float32)[od][None, :] * sig_alive   # [T,N]
    aeff = np.clip(aeff, 1e-12, 1.0 - 1e-7)
    logit = np.log(aeff / (1.0 - aeff)).astype(np.float32)          # [T,N]
    ck = (np.asarray(colors, np.float32)[0][od]
          * (1.0 - np.asarray(csg)[od].astype(np.float32))[:, None])  # [N,3]

    ys = ((np.arange(H) + 0.5) / H).astype(np.float32)
    x0, x1 = 0.5 / W, (W - 0.5) / W

    # --- active test per (t, n, row): exists x in [x0,x1] with all edges
    # arg = A x + (B gy_r + Cc) >= -MARGIN.  Chunked over T for memory.
    cnt = np.empty((T, NSLOT), np.int32)
    active = np.empty((T, N, NSLOT), bool)
    step = 32
    for t0 in range(0, T, step):
        sl = slice(t0, t0 + step)
        D = B[sl, :, :, None] * ys[None, None, None, :] + Cc[sl, :, :, None]
        Ae = A[sl, :, :, None]
        Asafe = np.where(Ae == 0, 1.0, Ae)
        lo = np.where(Ae > 0, (-MARGIN - D) / Asafe, x0)
        hi = np.where(Ae < 0, (-MARGIN - D) / Asafe, x1)
        lo = np.where((Ae == 0) & (D < -MARGIN), x1 + 1.0, lo)
        LO = np.maximum(x0, lo.max(axis=2))
        HI = np.minimum(x1, hi.min(axis=2))
        act = LO <= HI
        active[sl] = act
        cnt[sl] = act.sum(axis=1)

    # --- slots: rows sorted ascending by count; adaptive groups
    order = np.argsort(cnt, axis=1, kind="stable")       # [T, NSLOT]
    scnt = np.take_along_axis(cnt, order, axis=1)
    capr = scnt.max(axis=0)                              # [NSLOT]
    groups, NBTOT, CKTOT = _make_groups(capr)

    # active prim indices per (t, row), z-order preserved
    Cmax = max(g["C"] for g in groups)
    act_tr = np.transpose(active, (0, 2, 1))             # [T, R, N]
    idx = np.argsort(~act_tr, axis=2, kind="stable")[:, :, :Cmax]  # [T,R,Cmax]
    valid = np.take_along_axis(act_tr, idx, axis=2)      # [T,R,Cmax]

    Ah, Al = _split2(A)
    w4 = np.zeros((T, 4, NBTOT * BANK), fp16)
    ck2a = np.zeros((T, 128, max(CKTOT, 1)), fp16)
    tt = np.arange(T)[:, None, None]
    for g in groups:
        C, off, n_g, cpitch, spc, ckoff = (g["C"], g["off"], g["n"],
                                           g["cpitch"], g["spc"], g["ckoff"])
        rows = order[:, g["s0"]:g["s0"] + n_g]            # [T, n_g]
        pidx = np.take_along_axis(idx, rows[:, :, None], axis=1)[:, :, :C]
        pval = np.take_along_axis(valid, rows[:, :, None], axis=1)[:, :, :C]
        Ah_g = Ah[tt, pidx].astype(np.float32)            # [T,n,C,K]
        Al_g = Al[tt, pidx].astype(np.float32)
        B_g = B[tt, pidx]
        Cc_g = Cc[tt, pidx]
        gy_r = ys[rows][:, :, None, None]
        Cp = (B_g * gy_r + Cc_g).astype(np.float32)
        Cph, Cpl = _split2(Cp)
        lg = logit[tt, pidx].astype(np.float32)           # [T,n,C]
        lg = np.where(pval, lg, -30.0)
        lgh, lgl = _split2(lg)
        dead = ~pval[..., None]
        Ah_g = np.where(dead, 0.0, Ah_g)
        Al_g = np.where(dead, 0.0, Al_g)
        Cph = np.where(dead, fp16(0), Cph)
        Cpl = np.where(dead, fp16(0), Cpl)

        blk = np.zeros((T, n_g, C, PITCH, 4), fp16)
        blk[..., 0, 2] = lgh                    # x col: C' = logit
        blk[..., 0, 3] = lgl
        blk[..., 1:1 + K, 0] = Ah_g
        blk[..., 1:1 + K, 1] = Al_g
        blk[..., 1:1 + K, 2] = Cph
        blk[..., 1:1 + K, 3] = Cpl
        flat = blk.reshape(T, n_g * C * PITCH, 4)
        w4[:, :, off:off + flat.shape[1]] = flat.transpose(0, 2, 1)

        ckg = ck[pidx] * pval[..., None]                  # [T,n,C,3]
        for s in range(n_g):
            part0 = (s % spc) * cpitch
            ck2a[:, part0:part0 + C, ckoff + 3 * s:ckoff + 3 * s + 3] = \
                ckg[:, s]

    gx = ((np.arange(W) + 0.5) / W).astype(fp16)          # exact in fp16
    G4 = np.zeros((4, 128), fp16)
    G4[0] = gx
    G4[1] = gx
    G4[2] = 1.0
    G4[3] = 1.0
    ident = np.eye(128, dtype=fp16)

    return dict(w4=w4, ck2a=ck2a, G4=G4, ident=ident, groups=groups,
                nbtot=NBTOT, cktot=max(CKTOT, 1), order=order)


def _host_prep(trajectory, colors, alpha, z, csg):
    plan = _plan(trajectory, alpha, z, csg, colors)
    in_maps = []
    for c in range(N_CORES):
        fr = slice(c * F, (c + 1) * F)
        in_maps.append({
            "g4": np.ascontiguousarray(plan["G4"]),
            "ident": np.ascontiguousarray(plan["ident"]),
            "w4": np.ascontiguousarray(plan["w4"][fr]),
            "ck2a": np.ascontiguousarray(plan["ck2a"][fr]),
        })
    return in_maps, plan


# ---------------------------------------------------------------------------
# device program
# ---------------------------------------------------------------------------

def _build_nc(n_frames, groups, nbtot, cktot):
    import concourse.bass as bass
    import concourse.bacc as bacc
    import concourse.tile as tile
    from concourse import mybir
    from contextlib import ExitStack

    dt = mybir.dt
    AF = mybir.ActivationFunctionType
    ALU = mybir.AluOpType

    nc = bacc.Bacc(None)
    g4_d = nc.dram_tensor("g4", [4, 128], dt.float16, kind="ExternalInput")
    ident_d = nc.dram_tensor("ident", [128, 128], dt.float16,
                             kind="ExternalInput")
    w4_d = nc.dram_tensor("w4", [n_frames, 4, nbtot * BANK], dt.float16,
                          kind="ExternalInput")
    ck2a_d = nc.dram_tensor("ck2a", [n_frames, 128, cktot], dt.float16,
                            kind="ExternalInput")
    out_d = nc.dram_tensor("out", [n_frames, NSLOT, 128, 3], dt.float32,
                           kind="ExternalOutput")

    with tile.TileContext(nc) as tc:
        with ExitStack() as ctx:
            singles = ctx.enter_context(tc.tile_pool(name="singles", bufs=1))
            w4_pool = ctx.enter_context(tc.tile_pool(name="w4", bufs=3))
            ck_pool = ctx.enter_context(tc.tile_pool(name="ck", bufs=2))
            sp_pool = ctx.enter_context(tc.tile_pool(name="sp", bufs=3))
            cov_pool = ctx.enter_context(tc.tile_pool(name="cov", bufs=3))
            om_pool = ctx.enter_context(tc.tile_pool(name="om", bufs=3))
            tt_pool = ctx.enter_context(tc.tile_pool(name="tt", bufs=3))
            w_pool = ctx.enter_context(tc.tile_pool(name="w", bufs=3))
            wt_pool = ctx.enter_context(tc.tile_pool(name="wt", bufs=3))
            fb_pool = ctx.enter_context(tc.tile_pool(name="fb", bufs=2))
            s_psum = ctx.enter_context(
                tc.tile_pool(name="s_ps", bufs=2, space="PSUM"))
            t_psum = ctx.enter_context(
                tc.tile_pool(name="t_ps", bufs=2, space="PSUM"))
            c_psum = ctx.enter_context(
                tc.tile_pool(name="c_ps", bufs=2, space="PSUM"))

            g4_sb = singles.tile([4, 128], dt.float16)
            nc.sync.dma_start(out=g4_sb, in_=g4_d[:])
            ones16 = singles.tile([128, 1], dt.float16)
            nc.vector.memset(ones16, 1.0)
            ident_sb = singles.tile([128, 128], dt.float16)
            nc.sync.dma_start(out=ident_sb, in_=ident_d[:])
            maxgc = max(PITCH * g["C"] * g["n"] for g in groups)
            d1a_full = singles.tile([128, maxgc], dt.float16)
            nc.vector.memset(d1a_full, 0.0)
            nrep = (maxgc + PITCH - 1) // PITCH
            rcols = bass.AP(tensor=d1a_full.tensor,
                            offset=d1a_full.offset + PITCH - 1,
                            ap=[d1a_full.ap[0], [PITCH, nrep - 1], [1, 1]])
            nc.vector.memset(rcols, 1.0)
            d1b = {}
            for gi, g in enumerate(groups):
                C, n_g = g["C"], g["n"]
                t2 = singles.tile([128, n_g * (C + 1)], dt.float16,
                                  tag=f"d1b{gi}")
                nc.vector.memset(t2, 0.0)
                r2 = bass.AP(tensor=t2.tensor, offset=t2.offset,
                             ap=[t2.ap[0], [C + 1, n_g], [1, 1]])
                nc.vector.memset(r2, 1.0)
                d1b[gi] = t2

            for t in range(n_frames):
                ck_sb = ck_pool.tile([128, cktot], dt.float16, tag="ck")
                nc.sync.dma_start(out=ck_sb, in_=ck2a_d[t])
                co_ps = c_psum.tile([128, BANK], dt.float32, tag="co")
                for gi, g in enumerate(groups):
                    C, off, nb, n_g = g["C"], g["off"], g["nb"], g["n"]
                    cpitch, spc, ckoff = g["cpitch"], g["spc"], g["ckoff"]
                    gcols = nb * BANK
                    w4_sb = w4_pool.tile([4, gcols], dt.float16, tag="w4")
                    dmae = nc.sync if gi % 2 == 0 else nc.gpsimd
                    dmae.dma_start(out=w4_sb,
                                   in_=w4_d[t, :, off:off + gcols])
                    sp_sb = sp_pool.tile([128, gcols], dt.float16, tag="sp")
                    used = PITCH * C * n_g
                    for b0 in range(0, nb, 2):
                        nbk = min(2, nb - b0)
                        s_ps = s_psum.tile([128, 2 * BANK], dt.float32, tag="s")
                        for b in range(nbk):
                            nc.tensor.matmul(
                                s_ps[:, b * BANK:(b + 1) * BANK],
                                lhsT=g4_sb,
                                rhs=w4_sb[:, (b0 + b) * BANK:(b0 + b + 1) * BANK],
                                start=True, stop=True)
                        ncols = min(nbk * BANK, used - b0 * BANK)
                        nc.scalar.activation(
                            sp_sb[:, b0 * BANK:b0 * BANK + ncols],
                            s_ps[:, :ncols], AF.Sigmoid)
                    cov_sb = cov_pool.tile([128, PITCH * C * n_g], dt.float16,
                                           tag="cov")
                    glen = PITCH * C * n_g
                    nc.vector.tensor_tensor_scan(
                        out=cov_sb,
                        data0=sp_sb[:, :glen],
                        data1=d1a_full[:, :glen],
                        initial=ones16[:, 0:1],
                        op0=ALU.mult, op1=ALU.max)
                    om_sb = om_pool.tile([128, n_g * (C + 1)], dt.float16,
                                         tag="om")
                    r2 = bass.AP(tensor=om_sb.tensor, offset=om_sb.offset,
                                 ap=[om_sb.ap[0], [C + 1, n_g], [1, 1]])
                    nc.vector.memset(r2, 1.0)
                    a_ap = bass.AP(tensor=cov_sb.tensor,
                                   offset=cov_sb.offset + PITCH - 2,
                                   ap=[cov_sb.ap[0], [PITCH * C, n_g],
                                       [PITCH, C]])
                    om_ap = bass.AP(tensor=om_sb.tensor,
                                    offset=om_sb.offset + 1,
                                    ap=[om_sb.ap[0], [C + 1, n_g], [1, C]])
                    nc.vector.tensor_scalar(om_ap, a_ap, -1.0, 1.0,
                                            ALU.mult, ALU.add)
                    tt_sb = tt_pool.tile([128, n_g * (C + 1)], dt.float16,
                                         tag="tt")
                    nc.vector.tensor_tensor_scan(
                        out=tt_sb, data0=om_sb, data1=d1b[gi],
                        initial=ones16[:, 0:1], op0=ALU.mult, op1=ALU.max)
                    w_sb = w_pool.tile([128, n_g * cpitch], dt.float16, tag="w")
                    if cpitch > C:
                        pad = bass.AP(tensor=w_sb.tensor,
                                      offset=w_sb.offset + C,
                                      ap=[w_sb.ap[0], [cpitch, n_g],
                                          [1, cpitch - C]])
                        nc.gpsimd.memset(pad, 0.0)
                    t0_ap = bass.AP(tensor=tt_sb.tensor, offset=tt_sb.offset,
                                    ap=[tt_sb.ap[0], [C + 1, n_g], [1, C]])
                    t1_ap = bass.AP(tensor=tt_sb.tensor,
                                    offset=tt_sb.offset + 1,
                                    ap=[tt_sb.ap[0], [C + 1, n_g], [1, C]])
                    w_ap = bass.AP(tensor=w_sb.tensor, offset=w_sb.offset,
                                   ap=[w_sb.ap[0], [cpitch, n_g], [1, C]])
                    nc.vector.tensor_tensor(w_ap, t0_ap, t1_ap, ALU.subtract)
                    nchunk = (n_g + spc - 1) // spc
                    for j in range(nchunk):
                        ns = min(spc, n_g - j * spc)
                        ccols = ns * cpitch
                        wt_ps = t_psum.tile([128, 1024], dt.float16, tag="wt")
                        nc.tensor.transpose(
                            wt_ps[:ccols, :128],
                            w_sb[:, j * spc * cpitch:j * spc * cpitch + ccols],
                            ident_sb)
                        wt_sb = wt_pool.tile([128, 128], dt.float16, tag="wts")
                        cpeng = nc.vector if (gi + j) % 8 == 0 else nc.scalar
                        if cpeng is nc.vector:
                            cpeng.tensor_copy(wt_sb[:ccols, :],
                                              wt_ps[:ccols, :128])
                        else:
                            cpeng.copy(wt_sb[:ccols, :],
                                       wt_ps[:ccols, :128])
                        kk = ns * cpitch
                        for p in range(ns):
                            s = j * spc + p
                            slot = g["s0"] + s
                            nc.tensor.matmul(
                                co_ps[:, slot * 3:slot * 3 + 3],
                                lhsT=wt_sb[0:kk, :],
                                rhs=ck_sb[0:kk,
                                          ckoff + 3 * s:ckoff + 3 * s + 3],
                                start=True, stop=True)
                fb_sb = fb_pool.tile([128, NSLOT * 3], dt.float32, tag="fb")
                nc.scalar.copy(fb_sb, co_ps[:, :NSLOT * 3])
                src = fb_sb.rearrange("p (s ch) -> p s ch", ch=3)
                dst = out_d[t].rearrange("s p ch -> p s ch")
                nc.sync.dma_start(out=dst, in_=src)
    nc.finalize()
    return nc


def _get_program(n_frames, groups, nbtot, cktot):
    key = (n_frames, _groups_key(groups), nbtot, cktot)
    if key not in _CACHE:
        _CACHE[key] = _build_nc(n_frames, groups, nbtot, cktot)
    return _CACHE[key]


def _enable_jax_cache():
    try:
        import jax
        if jax.config.jax_compilation_cache_dir is None:
            jax.config.update("jax_compilation_cache_dir", "/tmp/jax_bass_cache")
            jax.config.update("jax_persistent_cache_min_entry_size_bytes", -1)
            jax.config.update("jax_persistent_cache_min_compile_time_secs", 0.5)
    except Exception:
        pass


def kernel(trajectory, colors, alpha, z, csg):
    from concourse.bass_utils import run_bass_kernel_spmd

    _enable_jax_cache()

    in_maps, plan = _host_prep(
        np.asarray(trajectory), np.asarray(colors), np.asarray(alpha),
        np.asarray(z), np.asarray(csg))
    nc = _get_program(F, plan["groups"], plan["nbtot"], plan["cktot"])
    res = run_bass_kernel_spmd(nc, in_maps, core_ids=list(range(N_CORES)))
    outs = [res.results[c]["out"] for c in range(N_CORES)]
    dev = np.concatenate(outs, axis=0)          # [192, slot, pix, 3]
    video = np.empty((T_TOTAL, H, W, 3), np.float32)
    order = plan["order"]
    for t in range(T_TOTAL):
        video[t, order[t]] = dev[t]
    return video[None].astype(np.float32)


if __name__ == "__main__":
    import time
    d = np.load("/root/problem/ref_cache.npz")
    t0 = time.time()
    in_maps, plan = _host_prep(d["trajectory"], d["colors"], d["alpha"],
                               d["z"], d["csg"])
    print(f"host prep: {time.time()-t0:.1f}s nbtot={plan['nbtot']}")
    print("groups:", _groups_key(plan["groups"]))
    t0 = time.time()
    nc = _build_nc(2, plan["groups"], plan["nbtot"], plan["cktot"])
    print(f"build 2f: {time.time()-t0:.1f}s")



# revision 8
# speedup vs baseline: 1.8316x; 1.8316x over previous
"""Trainium2 Bass kernel for nn_CBAE_EndToEnd — 2D-tile active-prim
compaction, segmented-product design (v2).

Each 8x16-pixel tile (128 pixels on partitions) only intersects ~7 of
the 128 primitives (exact per-row x-interval test with sigmoid
saturation margin, OR'd over the tile's 8 rows and intersected with the
tile's x-range).  Host packs, per (frame, tile), the active prims into
a pitch-13 fp16 matmul stream per prim:
  [x-col: logit(aeff) | e0..e11 edge cols]
with contract-6 lhsT [ox, ox, oy, oy, 1, 1] (within-tile pixel offsets,
exact in fp16) and per-column coefficient rows [Ah, Al, Bh, Bl, Cth,
Ctl]; the tile corner is folded into Ct so the lhsT is static across
all tiles/frames.  sigma(x-col) = aeff folds opacity into the product.

Device per group (tiles sorted by active count, uniform capacity C):
  PE    : arg = A*ox + B*oy + Ct via contract-6 fp16 matmul, static lhsT.
  ACT   : sigmoid over 2-PSUM-bank batches.
  DVE   : segmented product a = prod_13 sigma via tensor_reduce(mult)
          (1 elem/cycle vs 2 for the old scan); per-group compositing
          scan over (C+1)-pitch om; everything else moved off DVE.
  Pool  : om = 1-a (strided), w = t0-t1 subtract, PSUM->SBUF copies of
          transposed w, pad memsets.
  PE    : batched fp16 transposes of w (128-col chunks), one 3*ns-col
          color matmul per chunk into a shared PSUM bank per frame.
Output [frame, slot, pix, 3]; host un-permutes slots back to tiles.
"""

import numpy as np

H = 128
W = 128
N = 128
K = 12
SOFT = 0.01
T_TOTAL = 192
N_CORES = 8
F = T_TOTAL // N_CORES
MARGIN = 9.0           # |arg| beyond this counts as saturated
PITCH = K + 1          # x-col + 12 edges (no reset col needed)
TY, TX = 8, 16         # tile shape in pixels
NTY, NTX = H // TY, W // TX
NSLOT = NTY * NTX      # 128 tiles, one slot each
BANK = 512             # fp32 cols per PSUM bank
GLIM = 4096            # max matmul-stream cols per group (8 banks)

fp16 = np.float16

_CACHE = {}


# ---------------------------------------------------------------------------
# host prep
# ---------------------------------------------------------------------------

def _split2(x):
    x = np.asarray(x, np.float32)
    h = x.astype(fp16)
    l = (x - h.astype(np.float32)).astype(fp16)
    return h, l


def _make_groups(capr):
    """Greedy grouping of sorted slots: uniform cap per group, bounded
    column footprint.  capr[r] = max over frames of r-th smallest count."""
    groups = []
    s = 0
    col_off = 0
    ck_off = 0
    while s < NSLOT:
        n = 1
        while s + n < NSLOT and n < 16:
            cap = max(1, int(capr[s + n]))
            if PITCH * cap * (n + 1) > GLIM:
                break
            n += 1
        C = max(1, int(capr[s + n - 1]))
        cols = PITCH * C * n
        nb = (cols + BANK - 1) // BANK
        groups.append(dict(s0=s, n=n, C=C, off=col_off, nb=nb,
                           spc=max(1, 128 // C), ckoff=ck_off))
        col_off += nb * BANK
        ck_off += 3 * n
        s += n
    return groups, col_off // BANK, ck_off


def _groups_key(groups):
    return tuple((g["s0"], g["n"], g["C"]) for g in groups)


def _plan(trajectory, alpha, z, csg, colors):
    """Compute compaction plan + packed per-frame data for ALL frames."""
    T = trajectory.shape[0]
    od = np.argsort(z, kind="stable")[::-1]     # descending z = paint order
    traj = np.asarray(trajectory, np.float32)[:, 0, :]
    P = traj[:, : N * K * 2].reshape(T, N, K, 2)[:, od]
    alive = traj[:, N * K * 2:][:, od]
    v0 = P
    v1 = np.roll(P, -1, axis=2)
    e = v1 - v0
    area2 = np.sum(v0[..., 0] * v1[..., 1] - v1[..., 0] * v0[..., 1], axis=2)
    orient = np.sign(area2).astype(np.float32)[:, :, None]
    A = (-orient * e[..., 1] / SOFT).astype(np.float32)       # [T,N,K] gx coef
    B = (orient * e[..., 0] / SOFT).astype(np.float32)        # gy coef
    Cc = (orient * (e[..., 1] * v0[..., 0] - e[..., 0] * v0[..., 1]) / SOFT
          ).astype(np.float32)

    sig_alive = 1.0 / (1.0 + np.exp(-alive.astype(np.float32)))
    aeff = np.asarray(alpha, np.float32)[od][None, :] * sig_alive   # [T,N]
    aeff = np.clip(aeff, 1e-12, 1.0 - 1e-7)
    logit = np.log(aeff / (1.0 - aeff)).astype(np.float32)          # [T,N]
    ck = (np.asarray(colors, np.float32)[0][od]
          * (1.0 - np.asarray(csg)[od].astype(np.float32))[:, None])  # [N,3]

    ys = ((np.arange(H) + 0.5) / H).astype(np.float32)
    x0, x1 = 0.5 / W, (W - 0.5) / W

    # --- per-row exact feasible x-interval, then per-tile activity:
    # a prim is active in a tile iff some row of the tile has a feasible
    # x-interval intersecting the tile's x-range.
    cx0 = ((np.arange(NTX) * TX + 0.5) / W).astype(np.float32)
    cx1 = ((np.arange(NTX) * TX + TX - 0.5) / W).astype(np.float32)
    cnt = np.empty((T, NSLOT), np.int32)
    active = np.empty((T, N, NSLOT), bool)
    step = 32
    for t0 in range(0, T, step):
        sl = slice(t0, t0 + step)
        D = B[sl, :, :, None] * ys[None, None, None, :] + Cc[sl, :, :, None]
        Ae = A[sl, :, :, None]
        Asafe = np.where(Ae == 0, 1.0, Ae)
        lo = np.where(Ae > 0, (-MARGIN - D) / Asafe, x0)
        hi = np.where(Ae < 0, (-MARGIN - D) / Asafe, x1)
        lo = np.where((Ae == 0) & (D < -MARGIN), x1 + 1.0, lo)
        LO = np.maximum(x0, lo.max(axis=2))       # [t,N,H]
        HI = np.minimum(x1, hi.min(axis=2))
        LOr = LO.reshape(-1, N, NTY, TY)
        HIr = HI.reshape(-1, N, NTY, TY)
        act = (np.maximum(LOr[..., None], cx0[None, None, None, None, :])
               <= np.minimum(HIr[..., None], cx1[None, None, None, None, :])
               ).any(axis=3)                      # [t,N,NTY,NTX]
        act = act.reshape(-1, N, NSLOT)
        active[sl] = act
        cnt[sl] = act.sum(axis=1)

    # --- slots: tiles sorted ascending by count; adaptive groups
    order = np.argsort(cnt, axis=1, kind="stable")       # [T, NSLOT]
    scnt = np.take_along_axis(cnt, order, axis=1)
    capr = scnt.max(axis=0)                              # [NSLOT]
    groups, NBTOT, CKTOT = _make_groups(capr)

    # active prim indices per (t, tile), z-order preserved
    Cmax = max(g["C"] for g in groups)
    act_tr = np.transpose(active, (0, 2, 1))             # [T, S, N]
    idx = np.argsort(~act_tr, axis=2, kind="stable")[:, :, :Cmax]  # [T,S,Cmax]
    valid = np.take_along_axis(act_tr, idx, axis=2)      # [T,S,Cmax]

    Ah, Al = _split2(A)
    Bh, Bl = _split2(B)
    w6 = np.zeros((T, 6, NBTOT * BANK), fp16)
    ck2a = np.zeros((T, 128, max(CKTOT, 1)), fp16)
    tt = np.arange(T)[:, None, None]
    for g in groups:
        C, off, n_g, spc, ckoff = (g["C"], g["off"], g["n"],
                                   g["spc"], g["ckoff"])
        slots = order[:, g["s0"]:g["s0"] + n_g]           # [T, n_g] tile ids
        pidx = np.take_along_axis(idx, slots[:, :, None], axis=1)[:, :, :C]
        pval = np.take_along_axis(valid, slots[:, :, None], axis=1)[:, :, :C]
        Ah_g = Ah[tt, pidx].astype(np.float32)            # [T,n,C,K]
        Al_g = Al[tt, pidx].astype(np.float32)
        Bh_g = Bh[tt, pidx].astype(np.float32)
        Bl_g = Bl[tt, pidx].astype(np.float32)
        A_g = A[tt, pidx]
        B_g = B[tt, pidx]
        Cc_g = Cc[tt, pidx]
        tx0s = ((slots % NTX) * TX / W).astype(np.float32)[:, :, None, None]
        ty0s = ((slots // NTX) * TY / H).astype(np.float32)[:, :, None, None]
        Ct = (A_g * tx0s + B_g * ty0s + Cc_g).astype(np.float32)
        Cth, Ctl = _split2(Ct)
        lg = logit[tt, pidx].astype(np.float32)           # [T,n,C]
        lg = np.where(pval, lg, -30.0)
        lgh, lgl = _split2(lg)
        dead = ~pval[..., None]
        Ah_g = np.where(dead, 0.0, Ah_g)
        Al_g = np.where(dead, 0.0, Al_g)
        Bh_g = np.where(dead, 0.0, Bh_g)
        Bl_g = np.where(dead, 0.0, Bl_g)
        Cth = np.where(dead, fp16(0), Cth)
        Ctl = np.where(dead, fp16(0), Ctl)

        blk = np.zeros((T, n_g, C, PITCH, 6), fp16)
        blk[..., 0, 4] = lgh                    # x col: Ct = logit
        blk[..., 0, 5] = lgl
        blk[..., 1:1 + K, 0] = Ah_g
        blk[..., 1:1 + K, 1] = Al_g
        blk[..., 1:1 + K, 2] = Bh_g
        blk[..., 1:1 + K, 3] = Bl_g
        blk[..., 1:1 + K, 4] = Cth
        blk[..., 1:1 + K, 5] = Ctl
        flat = blk.reshape(T, n_g * C * PITCH, 6)
        w6[:, :, off:off + flat.shape[1]] = flat.transpose(0, 2, 1)

        ckg = ck[pidx] * pval[..., None]                  # [T,n,C,3]
        for s in range(n_g):
            part0 = (s % spc) * C
            ck2a[:, part0:part0 + C, ckoff + 3 * s:ckoff + 3 * s + 3] = \
                ckg[:, s]

    # static lhsT: within-tile pixel offsets (exact in fp16)
    p = np.arange(128)
    ox = ((p % TX + 0.5) / W).astype(fp16)
    oy = ((p // TX + 0.5) / H).astype(fp16)
    G6 = np.zeros((6, 128), fp16)
    G6[0] = ox
    G6[1] = ox
    G6[2] = oy
    G6[3] = oy
    G6[4] = 1.0
    G6[5] = 1.0
    ident = np.eye(128, dtype=fp16)

    return dict(w6=w6, ck2a=ck2a, G6=G6, ident=ident, groups=groups,
                nbtot=NBTOT, cktot=max(CKTOT, 1), order=order)


def _host_prep(trajectory, colors, alpha, z, csg):
    plan = _plan(trajectory, alpha, z, csg, colors)
    in_maps = []
    for c in range(N_CORES):
        fr = slice(c * F, (c + 1) * F)
        in_maps.append({
            "g6": np.ascontiguousarray(plan["G6"]),
            "ident": np.ascontiguousarray(plan["ident"]),
            "w6": np.ascontiguousarray(plan["w6"][fr]),
            "ck2a": np.ascontiguousarray(plan["ck2a"][fr]),
        })
    return in_maps, plan


# ---------------------------------------------------------------------------
# device program
# ---------------------------------------------------------------------------

def _build_nc(n_frames, groups, nbtot, cktot):
    import concourse.bass as bass
    import concourse.bacc as bacc
    import concourse.tile as tile
    from concourse import mybir
    from contextlib import ExitStack

    dt = mybir.dt
    AF = mybir.ActivationFunctionType
    ALU = mybir.AluOpType
    AX = mybir.AxisListType

    nc = bacc.Bacc(None)
    g6_d = nc.dram_tensor("g6", [6, 128], dt.float16, kind="ExternalInput")
    ident_d = nc.dram_tensor("ident", [128, 128], dt.float16,
                             kind="ExternalInput")
    w6_d = nc.dram_tensor("w6", [n_frames, 6, nbtot * BANK], dt.float16,
                          kind="ExternalInput")
    ck2a_d = nc.dram_tensor("ck2a", [n_frames, 128, cktot], dt.float16,
                            kind="ExternalInput")
    out_d = nc.dram_tensor("out", [n_frames, NSLOT, 128, 3], dt.float32,
                           kind="ExternalOutput")

    with tile.TileContext(nc) as tc:
        with ExitStack() as ctx:
            singles = ctx.enter_context(tc.tile_pool(name="singles", bufs=1))
            w6_pool = ctx.enter_context(tc.tile_pool(name="w6", bufs=3))
            ck_pool = ctx.enter_context(tc.tile_pool(name="ck", bufs=2))
            sp_pool = ctx.enter_context(tc.tile_pool(name="sp", bufs=3))
            a_pool = ctx.enter_context(tc.tile_pool(name="a", bufs=3))
            om_pool = ctx.enter_context(tc.tile_pool(name="om", bufs=3))
            tt_pool = ctx.enter_context(tc.tile_pool(name="tt", bufs=3))
            w_pool = ctx.enter_context(tc.tile_pool(name="w", bufs=3))
            wt_pool = ctx.enter_context(tc.tile_pool(name="wt", bufs=3))
            fb_pool = ctx.enter_context(tc.tile_pool(name="fb", bufs=2))
            s_psum = ctx.enter_context(
                tc.tile_pool(name="s_ps", bufs=2, space="PSUM"))
            t_psum = ctx.enter_context(
                tc.tile_pool(name="t_ps", bufs=2, space="PSUM"))
            c_psum = ctx.enter_context(
                tc.tile_pool(name="c_ps", bufs=2, space="PSUM"))

            g6_sb = singles.tile([6, 128], dt.float16)
            nc.sync.dma_start(out=g6_sb, in_=g6_d[:])
            ones16 = singles.tile([128, 1], dt.float16)
            nc.vector.memset(ones16, 1.0)
            ident_sb = singles.tile([128, 128], dt.float16)
            nc.sync.dma_start(out=ident_sb, in_=ident_d[:])
            d1b = {}
            for gi, g in enumerate(groups):
                C, n_g = g["C"], g["n"]
                t2 = singles.tile([128, n_g * (C + 1)], dt.float16,
                                  tag=f"d1b{gi}")
                nc.vector.memset(t2, 0.0)
                r2 = bass.AP(tensor=t2.tensor, offset=t2.offset,
                             ap=[t2.ap[0], [C + 1, n_g], [1, 1]])
                nc.vector.memset(r2, 1.0)
                d1b[gi] = t2

            for t in range(n_frames):
                ck_sb = ck_pool.tile([128, cktot], dt.float16, tag="ck")
                nc.sync.dma_start(out=ck_sb, in_=ck2a_d[t])
                co_ps = c_psum.tile([128, BANK], dt.float32, tag="co")
                for gi, g in enumerate(groups):
                    C, off, nb, n_g = g["C"], g["off"], g["nb"], g["n"]
                    spc, ckoff = g["spc"], g["ckoff"]
                    gcols = nb * BANK
                    w6_sb = w6_pool.tile([6, gcols], dt.float16, tag="w6")
                    dmae = nc.sync if gi % 2 == 0 else nc.gpsimd
                    dmae.dma_start(out=w6_sb,
                                   in_=w6_d[t, :, off:off + gcols])
                    sp_sb = sp_pool.tile([128, gcols], dt.float16, tag="sp")
                    used = PITCH * C * n_g
                    for b0 in range(0, nb, 2):
                        nbk = min(2, nb - b0)
                        s_ps = s_psum.tile([128, 2 * BANK], dt.float32, tag="s")
                        for b in range(nbk):
                            nc.tensor.matmul(
                                s_ps[:, b * BANK:(b + 1) * BANK],
                                lhsT=g6_sb,
                                rhs=w6_sb[:, (b0 + b) * BANK:(b0 + b + 1) * BANK],
                                start=True, stop=True)
                        ncols = min(nbk * BANK, used - b0 * BANK)
                        nc.scalar.activation(
                            sp_sb[:, b0 * BANK:b0 * BANK + ncols],
                            s_ps[:, :ncols], AF.Sigmoid)
                    # segmented product over pitch-13 segments
                    a_sb = a_pool.tile([128, C * n_g], dt.float16, tag="a")
                    sp_ap = bass.AP(tensor=sp_sb.tensor, offset=sp_sb.offset,
                                    ap=[sp_sb.ap[0], [PITCH, C * n_g],
                                        [1, PITCH]])
                    nc.vector.tensor_reduce(a_sb, sp_ap, AX.X, ALU.mult)
                    # om = 1 - a, with per-slot leading 1 for the scan reset
                    om_sb = om_pool.tile([128, n_g * (C + 1)], dt.float16,
                                         tag="om")
                    r2 = bass.AP(tensor=om_sb.tensor, offset=om_sb.offset,
                                 ap=[om_sb.ap[0], [C + 1, n_g], [1, 1]])
                    nc.gpsimd.memset(r2, 1.0)
                    om_ap = bass.AP(tensor=om_sb.tensor,
                                    offset=om_sb.offset + 1,
                                    ap=[om_sb.ap[0], [C + 1, n_g], [1, C]])
                    nc.gpsimd.tensor_scalar(om_ap, a_sb, -1.0, 1.0,
                                            ALU.mult, ALU.add)
                    tt_sb = tt_pool.tile([128, n_g * (C + 1)], dt.float16,
                                         tag="tt")
                    nc.vector.tensor_tensor_scan(
                        out=tt_sb, data0=om_sb, data1=d1b[gi],
                        initial=ones16[:, 0:1], op0=ALU.mult, op1=ALU.max)
                    w_sb = w_pool.tile([128, n_g * C], dt.float16, tag="w")
                    t0_ap = bass.AP(tensor=tt_sb.tensor, offset=tt_sb.offset,
                                    ap=[tt_sb.ap[0], [C + 1, n_g], [1, C]])
                    t1_ap = bass.AP(tensor=tt_sb.tensor,
                                    offset=tt_sb.offset + 1,
                                    ap=[tt_sb.ap[0], [C + 1, n_g], [1, C]])
                    nc.gpsimd.tensor_tensor(w_sb, t0_ap, t1_ap, ALU.subtract)
                    nchunk = (n_g + spc - 1) // spc
                    for j in range(nchunk):
                        ns = min(spc, n_g - j * spc)
                        ccols = ns * C
                        wt_ps = t_psum.tile([128, 1024], dt.float16, tag="wt")
                        nc.tensor.transpose(
                            wt_ps[:ccols, :128],
                            w_sb[:, j * spc * C:j * spc * C + ccols],
                            ident_sb)
                        wt_sb = wt_pool.tile([128, 128], dt.float16, tag="wts")
                        cpeng = nc.scalar if (gi + j) % 2 == 0 else nc.vector
                        if cpeng is nc.vector:
                            cpeng.tensor_copy(wt_sb[:ccols, :],
                                              wt_ps[:ccols, :128])
                        else:
                            cpeng.copy(wt_sb[:ccols, :],
                                       wt_ps[:ccols, :128])
                        s0 = j * spc
                        nc.tensor.matmul(
                            co_ps[:, (g["s0"] + s0) * 3:
                                  (g["s0"] + s0 + ns) * 3],
                            lhsT=wt_sb[0:ccols, :],
                            rhs=ck_sb[0:ccols,
                                      ckoff + 3 * s0:ckoff + 3 * (s0 + ns)],
                            start=True, stop=True)
                fb_sb = fb_pool.tile([128, NSLOT * 3], dt.float32, tag="fb")
                nc.scalar.copy(fb_sb, co_ps[:, :NSLOT * 3])
                src = fb_sb.rearrange("p (s ch) -> p s ch", ch=3)
                dst = out_d[t].rearrange("s p ch -> p s ch")
                nc.sync.dma_start(out=dst, in_=src)
    nc.finalize()
    return nc


def _get_program(n_frames, groups, nbtot, cktot):
    key = (n_frames, _groups_key(groups), nbtot, cktot)
    if key not in _CACHE:
        _CACHE[key] = _build_nc(n_frames, groups, nbtot, cktot)
    return _CACHE[key]


def _enable_jax_cache():
    try:
        import jax
        if jax.config.jax_compilation_cache_dir is None:
            jax.config.update("jax_compilation_cache_dir", "/tmp/jax_bass_cache")
            jax.config.update("jax_persistent_cache_min_entry_size_bytes", -1)
            jax.config.update("jax_persistent_cache_min_compile_time_secs", 0.5)
    except Exception:
        pass


def _unpermute(dev, order):
    """dev [T, slot, 128, 3] -> video [T, H, W, 3] (tile un-permute)."""
    T = dev.shape[0]
    video = np.empty((T, NSLOT, TY, TX, 3), np.float32)
    tt = np.arange(T)[:, None]
    video[tt, order] = dev.reshape(T, NSLOT, TY, TX, 3)
    video = video.reshape(T, NTY, NTX, TY, TX, 3)
    video = video.transpose(0, 1, 3, 2, 4, 5).reshape(T, H, W, 3)
    return video


def kernel(trajectory, colors, alpha, z, csg):
    from concourse.bass_utils import run_bass_kernel_spmd

    _enable_jax_cache()

    in_maps, plan = _host_prep(
        np.asarray(trajectory), np.asarray(colors), np.asarray(alpha),
        np.asarray(z), np.asarray(csg))
    nc = _get_program(F, plan["groups"], plan["nbtot"], plan["cktot"])
    res = run_bass_kernel_spmd(nc, in_maps, core_ids=list(range(N_CORES)))
    outs = [res.results[c]["out"] for c in range(N_CORES)]
    dev = np.concatenate(outs, axis=0)          # [192, slot, pix, 3]
    video = _unpermute(dev, plan["order"])
    return video[None].astype(np.float32)


if __name__ == "__main__":
    import time
    d = np.load("/root/problem/ref_cache.npz")
    t0 = time.time()
    in_maps, plan = _host_prep(d["trajectory"], d["colors"], d["alpha"],
                               d["z"], d["csg"])
    print(f"host prep: {time.time()-t0:.1f}s nbtot={plan['nbtot']}")
    print("groups:", _groups_key(plan["groups"]))
    t0 = time.time()
    nc = _build_nc(2, plan["groups"], plan["nbtot"], plan["cktot"])
    print(f"build 2f: {time.time()-t0:.1f}s")


# revision 10
# speedup vs baseline: 2.8961x; 1.5811x over previous
"""Trainium2 Bass kernel for nn_CBAE_EndToEnd — 2D-tile active-prim
compaction, segmented-product design (v2).

Each 8x16-pixel tile (128 pixels on partitions) only intersects ~7 of
the 128 primitives (exact per-row x-interval test with sigmoid
saturation margin, OR'd over the tile's 8 rows and intersected with the
tile's x-range).  Host packs, per (frame, tile), the active prims into
a pitch-13 fp16 matmul stream per prim:
  [x-col: logit(aeff) | e0..e11 edge cols]
with contract-6 lhsT [ox, ox, oy, oy, 1, 1] (within-tile pixel offsets,
exact in fp16) and per-column coefficient rows [Ah, Al, Bh, Bl, Cth,
Ctl]; the tile corner is folded into Ct so the lhsT is static across
all tiles/frames.  sigma(x-col) = aeff folds opacity into the product.

Device per group (tiles sorted by active count, uniform capacity C):
  PE    : arg = A*ox + B*oy + Ct via contract-6 fp16 matmul, static lhsT.
  ACT   : sigmoid over 2-PSUM-bank batches.
  DVE   : segmented product a = prod_13 sigma via tensor_reduce(mult)
          (1 elem/cycle vs 2 for the old scan); per-group compositing
          scan over (C+1)-pitch om; everything else moved off DVE.
  Pool  : om = 1-a (strided), w = t0-t1 subtract, PSUM->SBUF copies of
          transposed w, pad memsets.
  PE    : batched fp16 transposes of w (128-col chunks), one 3*ns-col
          color matmul per chunk into a shared PSUM bank per frame.
Output [frame, slot, pix, 3]; host un-permutes slots back to tiles.
"""

import numpy as np

H = 128
W = 128
N = 128
K = 12
SOFT = 0.01
T_TOTAL = 192
N_CORES = 8
F = T_TOTAL // N_CORES
MARGIN = 9.0           # |arg| beyond this counts as saturated
PITCH = K + 1          # x-col + 12 edges (no reset col needed)
TY, TX = 8, 16         # tile shape in pixels
NTY, NTX = H // TY, W // TX
NSLOT = NTY * NTX      # 128 tiles, one slot each
BANK = 512             # fp32 cols per PSUM bank
GLIM = 4096            # max matmul-stream cols per group (8 banks)

fp16 = np.float16

_CACHE = {}


# ---------------------------------------------------------------------------
# host prep
# ---------------------------------------------------------------------------

def _split2(x):
    x = np.asarray(x, np.float32)
    h = x.astype(fp16)
    l = (x - h.astype(np.float32)).astype(fp16)
    return h, l


def _make_groups(capr):
    """Greedy grouping of sorted slots: uniform cap per group, bounded
    column footprint.  capr[r] = max over frames of r-th smallest count."""
    groups = []
    s = 0
    col_off = 0
    ck_off = 0
    while s < NSLOT:
        n = 1
        while s + n < NSLOT and n < 16:
            cap = max(1, int(capr[s + n]))
            if PITCH * cap * (n + 1) > GLIM:
                break
            n += 1
        C = max(1, int(capr[s + n - 1]))
        cols = PITCH * C * n
        nb = (cols + BANK - 1) // BANK
        groups.append(dict(s0=s, n=n, C=C, off=col_off, nb=nb,
                           spc=max(1, 128 // C), ckoff=ck_off))
        col_off += nb * BANK
        ck_off += 3 * n
        s += n
    return groups, col_off // BANK, ck_off


def _groups_key(groups):
    return tuple((g["s0"], g["n"], g["C"]) for g in groups)


def _plan(trajectory, alpha, z, csg, colors):
    """Compute compaction plan + packed per-frame data for ALL frames."""
    T = trajectory.shape[0]
    od = np.argsort(z, kind="stable")[::-1]     # descending z = paint order
    traj = np.asarray(trajectory, np.float32)[:, 0, :]
    P = traj[:, : N * K * 2].reshape(T, N, K, 2)[:, od]
    alive = traj[:, N * K * 2:][:, od]
    v0 = P
    v1 = np.roll(P, -1, axis=2)
    e = v1 - v0
    area2 = np.sum(v0[..., 0] * v1[..., 1] - v1[..., 0] * v0[..., 1], axis=2)
    orient = np.sign(area2).astype(np.float32)[:, :, None]
    A = (-orient * e[..., 1] / SOFT).astype(np.float32)       # [T,N,K] gx coef
    B = (orient * e[..., 0] / SOFT).astype(np.float32)        # gy coef
    Cc = (orient * (e[..., 1] * v0[..., 0] - e[..., 0] * v0[..., 1]) / SOFT
          ).astype(np.float32)

    sig_alive = 1.0 / (1.0 + np.exp(-alive.astype(np.float32)))
    aeff = np.asarray(alpha, np.float32)[od][None, :] * sig_alive   # [T,N]
    aeff = np.clip(aeff, 1e-12, 1.0 - 1e-7)
    logit = np.log(aeff / (1.0 - aeff)).astype(np.float32)          # [T,N]
    ck = (np.asarray(colors, np.float32)[0][od]
          * (1.0 - np.asarray(csg)[od].astype(np.float32))[:, None])  # [N,3]

    ys = ((np.arange(H) + 0.5) / H).astype(np.float32)
    x0, x1 = 0.5 / W, (W - 0.5) / W

    # --- per-row exact feasible x-interval, then per-tile activity:
    # a prim is active in a tile iff some row of the tile has a feasible
    # x-interval intersecting the tile's x-range.
    cx0 = ((np.arange(NTX) * TX + 0.5) / W).astype(np.float32)
    cx1 = ((np.arange(NTX) * TX + TX - 0.5) / W).astype(np.float32)
    cnt = np.empty((T, NSLOT), np.int32)
    active = np.empty((T, N, NSLOT), bool)
    step = 32
    for t0 in range(0, T, step):
        sl = slice(t0, t0 + step)
        D = B[sl, :, :, None] * ys[None, None, None, :] + Cc[sl, :, :, None]
        Ae = A[sl, :, :, None]
        Asafe = np.where(Ae == 0, 1.0, Ae)
        lo = np.where(Ae > 0, (-MARGIN - D) / Asafe, x0)
        hi = np.where(Ae < 0, (-MARGIN - D) / Asafe, x1)
        lo = np.where((Ae == 0) & (D < -MARGIN), x1 + 1.0, lo)
        LO = np.maximum(x0, lo.max(axis=2))       # [t,N,H]
        HI = np.minimum(x1, hi.min(axis=2))
        LOr = LO.reshape(-1, N, NTY, TY)
        HIr = HI.reshape(-1, N, NTY, TY)
        act = (np.maximum(LOr[..., None], cx0[None, None, None, None, :])
               <= np.minimum(HIr[..., None], cx1[None, None, None, None, :])
               ).any(axis=3)                      # [t,N,NTY,NTX]
        act = act.reshape(-1, N, NSLOT)
        active[sl] = act
        cnt[sl] = act.sum(axis=1)

    # --- slots: tiles sorted ascending by count; adaptive groups
    order = np.argsort(cnt, axis=1, kind="stable")       # [T, NSLOT]
    scnt = np.take_along_axis(cnt, order, axis=1)
    capr = scnt.max(axis=0)                              # [NSLOT]
    groups, NBTOT, CKTOT = _make_groups(capr)

    # active prim indices per (t, tile), z-order preserved
    Cmax = max(g["C"] for g in groups)
    act_tr = np.transpose(active, (0, 2, 1))             # [T, S, N]
    idx = np.argsort(~act_tr, axis=2, kind="stable")[:, :, :Cmax]  # [T,S,Cmax]
    valid = np.take_along_axis(act_tr, idx, axis=2)      # [T,S,Cmax]

    Ah, Al = _split2(A)
    Bh, Bl = _split2(B)
    w6 = np.zeros((T, 6, NBTOT * BANK), fp16)
    ck2a = np.zeros((T, 128, max(CKTOT, 1)), fp16)
    tt = np.arange(T)[:, None, None]
    for g in groups:
        C, off, n_g, spc, ckoff = (g["C"], g["off"], g["n"],
                                   g["spc"], g["ckoff"])
        slots = order[:, g["s0"]:g["s0"] + n_g]           # [T, n_g] tile ids
        pidx = np.take_along_axis(idx, slots[:, :, None], axis=1)[:, :, :C]
        pval = np.take_along_axis(valid, slots[:, :, None], axis=1)[:, :, :C]
        Ah_g = Ah[tt, pidx].astype(np.float32)            # [T,n,C,K]
        Al_g = Al[tt, pidx].astype(np.float32)
        Bh_g = Bh[tt, pidx].astype(np.float32)
        Bl_g = Bl[tt, pidx].astype(np.float32)
        A_g = A[tt, pidx]
        B_g = B[tt, pidx]
        Cc_g = Cc[tt, pidx]
        tx0s = ((slots % NTX) * TX / W).astype(np.float32)[:, :, None, None]
        ty0s = ((slots // NTX) * TY / H).astype(np.float32)[:, :, None, None]
        Ct = (A_g * tx0s + B_g * ty0s + Cc_g).astype(np.float32)
        Cth, Ctl = _split2(Ct)
        lg = logit[tt, pidx].astype(np.float32)           # [T,n,C]
        lg = np.where(pval, lg, -30.0)
        lgh, lgl = _split2(lg)
        dead = ~pval[..., None]
        Ah_g = np.where(dead, 0.0, Ah_g)
        Al_g = np.where(dead, 0.0, Al_g)
        Bh_g = np.where(dead, 0.0, Bh_g)
        Bl_g = np.where(dead, 0.0, Bl_g)
        Cth = np.where(dead, fp16(0), Cth)
        Ctl = np.where(dead, fp16(0), Ctl)

        blk = np.zeros((T, n_g, C, PITCH, 6), fp16)
        blk[..., 0, 4] = lgh                    # x col: Ct = logit
        blk[..., 0, 5] = lgl
        blk[..., 1:1 + K, 0] = Ah_g
        blk[..., 1:1 + K, 1] = Al_g
        blk[..., 1:1 + K, 2] = Bh_g
        blk[..., 1:1 + K, 3] = Bl_g
        blk[..., 1:1 + K, 4] = Cth
        blk[..., 1:1 + K, 5] = Ctl
        flat = blk.reshape(T, n_g * C * PITCH, 6)
        w6[:, :, off:off + flat.shape[1]] = flat.transpose(0, 2, 1)

        ckg = ck[pidx] * pval[..., None]                  # [T,n,C,3]
        for s in range(n_g):
            part0 = (s % spc) * C
            ck2a[:, part0:part0 + C, ckoff + 3 * s:ckoff + 3 * s + 3] = \
                ckg[:, s]

    # static lhsT: within-tile pixel offsets (exact in fp16)
    p = np.arange(128)
    ox = ((p % TX + 0.5) / W).astype(fp16)
    oy = ((p // TX + 0.5) / H).astype(fp16)
    G6 = np.zeros((6, 128), fp16)
    G6[0] = ox
    G6[1] = ox
    G6[2] = oy
    G6[3] = oy
    G6[4] = 1.0
    G6[5] = 1.0
    ident = np.eye(128, dtype=fp16)

    return dict(w6=w6, ck2a=ck2a, G6=G6, ident=ident, groups=groups,
                nbtot=NBTOT, cktot=max(CKTOT, 1), order=order)


def _host_prep(trajectory, colors, alpha, z, csg):
    plan = _plan(trajectory, alpha, z, csg, colors)
    in_maps = []
    for c in range(N_CORES):
        fr = slice(c * F, (c + 1) * F)
        in_maps.append({
            "g6": np.ascontiguousarray(plan["G6"]),
            "ident": np.ascontiguousarray(plan["ident"]),
            "w6": np.ascontiguousarray(plan["w6"][fr]),
            "ck2a": np.ascontiguousarray(plan["ck2a"][fr]),
        })
    return in_maps, plan


# ---------------------------------------------------------------------------
# device program
# ---------------------------------------------------------------------------

def _build_nc(n_frames, groups, nbtot, cktot):
    import concourse.bass as bass
    import concourse.bacc as bacc
    import concourse.tile as tile
    from concourse import mybir
    from contextlib import ExitStack

    dt = mybir.dt
    AF = mybir.ActivationFunctionType
    ALU = mybir.AluOpType
    AX = mybir.AxisListType

    nc = bacc.Bacc(None)
    g6_d = nc.dram_tensor("g6", [6, 128], dt.float16, kind="ExternalInput")
    ident_d = nc.dram_tensor("ident", [128, 128], dt.float16,
                             kind="ExternalInput")
    w6_d = nc.dram_tensor("w6", [n_frames, 6, nbtot * BANK], dt.float16,
                          kind="ExternalInput")
    ck2a_d = nc.dram_tensor("ck2a", [n_frames, 128, cktot], dt.float16,
                            kind="ExternalInput")
    out_d = nc.dram_tensor("out", [n_frames, NSLOT, 128, 3], dt.float32,
                           kind="ExternalOutput")

    with tile.TileContext(nc) as tc:
        with ExitStack() as ctx:
            singles = ctx.enter_context(tc.tile_pool(name="singles", bufs=1))
            w6_pool = ctx.enter_context(tc.tile_pool(name="w6", bufs=3))
            ck_pool = ctx.enter_context(tc.tile_pool(name="ck", bufs=2))
            sp_pool = ctx.enter_context(tc.tile_pool(name="sp", bufs=3))
            a_pool = ctx.enter_context(tc.tile_pool(name="a", bufs=3))
            om_pool = ctx.enter_context(tc.tile_pool(name="om", bufs=3))
            tt_pool = ctx.enter_context(tc.tile_pool(name="tt", bufs=3))
            w_pool = ctx.enter_context(tc.tile_pool(name="w", bufs=3))
            wt_pool = ctx.enter_context(tc.tile_pool(name="wt", bufs=3))
            fb_pool = ctx.enter_context(tc.tile_pool(name="fb", bufs=2))
            s_psum = ctx.enter_context(
                tc.tile_pool(name="s_ps", bufs=2, space="PSUM"))
            t_psum = ctx.enter_context(
                tc.tile_pool(name="t_ps", bufs=2, space="PSUM"))
            c_psum = ctx.enter_context(
                tc.tile_pool(name="c_ps", bufs=2, space="PSUM"))

            g6_sb = singles.tile([6, 128], dt.float16)
            nc.sync.dma_start(out=g6_sb, in_=g6_d[:])
            ones16 = singles.tile([128, 1], dt.float16)
            nc.vector.memset(ones16, 1.0)
            ident_sb = singles.tile([128, 128], dt.float16)
            nc.sync.dma_start(out=ident_sb, in_=ident_d[:])
            d1b = {}
            for gi, g in enumerate(groups):
                C, n_g = g["C"], g["n"]
                t2 = singles.tile([128, n_g * (C + 1)], dt.float16,
                                  tag=f"d1b{gi}")
                nc.vector.memset(t2, 0.0)
                r2 = bass.AP(tensor=t2.tensor, offset=t2.offset,
                             ap=[t2.ap[0], [C + 1, n_g], [1, 1]])
                nc.vector.memset(r2, 1.0)
                d1b[gi] = t2

            for t in range(n_frames):
                ck_sb = ck_pool.tile([128, cktot], dt.float16, tag="ck")
                nc.sync.dma_start(out=ck_sb, in_=ck2a_d[t])
                co_ps = c_psum.tile([128, BANK], dt.float32, tag="co")
                for gi, g in enumerate(groups):
                    C, off, nb, n_g = g["C"], g["off"], g["nb"], g["n"]
                    spc, ckoff = g["spc"], g["ckoff"]
                    gcols = nb * BANK
                    w6_sb = w6_pool.tile([6, gcols], dt.float16, tag="w6")
                    nc.gpsimd.dma_start(out=w6_sb,
                                        in_=w6_d[t, :, off:off + gcols])
                    sp_sb = sp_pool.tile([128, gcols], dt.float16, tag="sp")
                    used = PITCH * C * n_g
                    for b0 in range(0, nb, 2):
                        nbk = min(2, nb - b0)
                        s_ps = s_psum.tile([128, 2 * BANK], dt.float32, tag="s")
                        for b in range(nbk):
                            nc.tensor.matmul(
                                s_ps[:, b * BANK:(b + 1) * BANK],
                                lhsT=g6_sb,
                                rhs=w6_sb[:, (b0 + b) * BANK:(b0 + b + 1) * BANK],
                                start=True, stop=True)
                        ncols = min(nbk * BANK, used - b0 * BANK)
                        nc.scalar.activation(
                            sp_sb[:, b0 * BANK:b0 * BANK + ncols],
                            s_ps[:, :ncols], AF.Sigmoid)
                    # segmented product over pitch-13 segments
                    a_sb = a_pool.tile([128, C * n_g], dt.float16, tag="a")
                    sp_ap = bass.AP(tensor=sp_sb.tensor, offset=sp_sb.offset,
                                    ap=[sp_sb.ap[0], [PITCH, C * n_g],
                                        [1, PITCH]])
                    nc.vector.tensor_reduce(a_sb, sp_ap, AX.X, ALU.mult)
                    # om = 1 - a, with per-slot leading 1 for the scan reset
                    om_sb = om_pool.tile([128, n_g * (C + 1)], dt.float16,
                                         tag="om")
                    r2 = bass.AP(tensor=om_sb.tensor, offset=om_sb.offset,
                                 ap=[om_sb.ap[0], [C + 1, n_g], [1, 1]])
                    nc.gpsimd.memset(r2, 1.0)
                    om_ap = bass.AP(tensor=om_sb.tensor,
                                    offset=om_sb.offset + 1,
                                    ap=[om_sb.ap[0], [C + 1, n_g], [1, C]])
                    nc.gpsimd.tensor_scalar(om_ap, a_sb, -1.0, 1.0,
                                            ALU.mult, ALU.add)
                    tt_sb = tt_pool.tile([128, n_g * (C + 1)], dt.float16,
                                         tag="tt")
                    nc.vector.tensor_tensor_scan(
                        out=tt_sb, data0=om_sb, data1=d1b[gi],
                        initial=ones16[:, 0:1], op0=ALU.mult, op1=ALU.max)
                    w_sb = w_pool.tile([128, n_g * C], dt.float16, tag="w")
                    t0_ap = bass.AP(tensor=tt_sb.tensor, offset=tt_sb.offset,
                                    ap=[tt_sb.ap[0], [C + 1, n_g], [1, C]])
                    t1_ap = bass.AP(tensor=tt_sb.tensor,
                                    offset=tt_sb.offset + 1,
                                    ap=[tt_sb.ap[0], [C + 1, n_g], [1, C]])
                    nc.gpsimd.tensor_tensor(w_sb, t0_ap, t1_ap, ALU.subtract)
                    nchunk = (n_g + spc - 1) // spc
                    for j in range(nchunk):
                        ns = min(spc, n_g - j * spc)
                        ccols = ns * C
                        wt_ps = t_psum.tile([128, 1024], dt.float16, tag="wt")
                        nc.tensor.transpose(
                            wt_ps[:ccols, :128],
                            w_sb[:, j * spc * C:j * spc * C + ccols],
                            ident_sb)
                        wt_sb = wt_pool.tile([128, 128], dt.float16, tag="wts")
                        nc.scalar.copy(wt_sb[:ccols, :],
                                       wt_ps[:ccols, :128])
                        s0 = j * spc
                        nc.tensor.matmul(
                            co_ps[:, (g["s0"] + s0) * 3:
                                  (g["s0"] + s0 + ns) * 3],
                            lhsT=wt_sb[0:ccols, :],
                            rhs=ck_sb[0:ccols,
                                      ckoff + 3 * s0:ckoff + 3 * (s0 + ns)],
                            start=True, stop=True)
                fb_sb = fb_pool.tile([128, NSLOT * 3], dt.float32, tag="fb")
                nc.scalar.copy(fb_sb, co_ps[:, :NSLOT * 3])
                src = fb_sb.rearrange("p (s ch) -> p s ch", ch=3)
                dst = out_d[t].rearrange("s p ch -> p s ch")
                nc.sync.dma_start(out=dst, in_=src)
    nc.finalize()
    return nc


def _get_program(n_frames, groups, nbtot, cktot):
    key = (n_frames, _groups_key(groups), nbtot, cktot)
    if key not in _CACHE:
        _CACHE[key] = _build_nc(n_frames, groups, nbtot, cktot)
    return _CACHE[key]


def _enable_jax_cache():
    try:
        import jax
        if jax.config.jax_compilation_cache_dir is None:
            jax.config.update("jax_compilation_cache_dir", "/tmp/jax_bass_cache")
            jax.config.update("jax_persistent_cache_min_entry_size_bytes", -1)
            jax.config.update("jax_persistent_cache_min_compile_time_secs", 0.5)
    except Exception:
        pass


def _unpermute(dev, order):
    """dev [T, slot, 128, 3] -> video [T, H, W, 3] (tile un-permute)."""
    T = dev.shape[0]
    video = np.empty((T, NSLOT, TY, TX, 3), np.float32)
    tt = np.arange(T)[:, None]
    video[tt, order] = dev.reshape(T, NSLOT, TY, TX, 3)
    video = video.reshape(T, NTY, NTX, TY, TX, 3)
    video = video.transpose(0, 1, 3, 2, 4, 5).reshape(T, H, W, 3)
    return video


def kernel(trajectory, colors, alpha, z, csg):
    from concourse.bass_utils import run_bass_kernel_spmd

    _enable_jax_cache()

    in_maps, plan = _host_prep(
        np.asarray(trajectory), np.asarray(colors), np.asarray(alpha),
        np.asarray(z), np.asarray(csg))
    nc = _get_program(F, plan["groups"], plan["nbtot"], plan["cktot"])
    res = run_bass_kernel_spmd(nc, in_maps, core_ids=list(range(N_CORES)))
    outs = [res.results[c]["out"] for c in range(N_CORES)]
    dev = np.concatenate(outs, axis=0)          # [192, slot, pix, 3]
    video = _unpermute(dev, plan["order"])
    return video[None].astype(np.float32)


if __name__ == "__main__":
    import time
    d = np.load("/root/problem/ref_cache.npz")
    t0 = time.time()
    in_maps, plan = _host_prep(d["trajectory"], d["colors"], d["alpha"],
                               d["z"], d["csg"])
    print(f"host prep: {time.time()-t0:.1f}s nbtot={plan['nbtot']}")
    print("groups:", _groups_key(plan["groups"]))
    t0 = time.time()
    nc = _build_nc(2, plan["groups"], plan["nbtot"], plan["cktot"])
    print(f"build 2f: {time.time()-t0:.1f}s")


# revision 13
# speedup vs baseline: 3.1322x; 1.0815x over previous
"""Trainium2 Bass kernel for nn_CBAE_EndToEnd — 2D-tile active-prim
compaction, segmented-product design (v2).

Each 8x16-pixel tile (128 pixels on partitions) only intersects ~7 of
the 128 primitives (exact per-row x-interval test with sigmoid
saturation margin, OR'd over the tile's 8 rows and intersected with the
tile's x-range).  Host packs, per (frame, tile), the active prims into
a pitch-13 fp16 matmul stream per prim:
  [x-col: logit(aeff) | e0..e11 edge cols]
with contract-6 lhsT [ox, ox, oy, oy, 1, 1] (within-tile pixel offsets,
exact in fp16) and per-column coefficient rows [Ah, Al, Bh, Bl, Cth,
Ctl]; the tile corner is folded into Ct so the lhsT is static across
all tiles/frames.  sigma(x-col) = aeff folds opacity into the product.

Device per group (tiles sorted by active count, uniform capacity C):
  PE    : arg = A*ox + B*oy + Ct via contract-6 fp16 matmul, static lhsT.
  ACT   : sigmoid over 2-PSUM-bank batches.
  DVE   : segmented product a = prod_13 sigma via tensor_reduce(mult)
          (1 elem/cycle vs 2 for the old scan); per-group compositing
          scan over (C+1)-pitch om; everything else moved off DVE.
  Pool  : om = 1-a (strided), w = t0-t1 subtract, PSUM->SBUF copies of
          transposed w, pad memsets.
  PE    : batched fp16 transposes of w (128-col chunks), one 3*ns-col
          color matmul per chunk into a shared PSUM bank per frame.
Output [frame, slot, pix, 3]; host un-permutes slots back to tiles.
"""

import numpy as np

H = 128
W = 128
N = 128
K = 12
SOFT = 0.01
T_TOTAL = 192
N_CORES = 8
F = T_TOTAL // N_CORES
MARGIN = 9.0           # |arg| beyond this counts as saturated
PITCH = K + 1          # x-col + 12 edges (no reset col needed)
TY, TX = 8, 16         # tile shape in pixels
NTY, NTX = H // TY, W // TX
NSLOT = NTY * NTX      # 128 tiles, one slot each
BANK = 512             # fp32 cols per PSUM bank
GLIM = 4096            # max matmul-stream cols per group (8 banks)

fp16 = np.float16

_CACHE = {}


# ---------------------------------------------------------------------------
# host prep
# ---------------------------------------------------------------------------

def _split2(x):
    x = np.asarray(x, np.float32)
    h = x.astype(fp16)
    l = (x - h.astype(np.float32)).astype(fp16)
    return h, l


def _make_groups(capr, ovh=400):
    """DP-optimal grouping of sorted slots: uniform cap per group.
    capr[r] = max over frames of r-th smallest count.  Minimizes
    bank-rounded stream columns + a per-group fixed overhead."""
    NS = len(capr)
    INF = float("inf")
    cost = [0.0] + [INF] * NS
    prev = [0] * (NS + 1)
    for i in range(1, NS + 1):
        C = max(1, int(capr[i - 1]))
        for j in range(i - 1, -1, -1):
            cols = PITCH * C * (i - j)
            if cols > GLIM:
                break
            nb = (cols + BANK - 1) // BANK
            c = cost[j] + nb * BANK + ovh
            if c < cost[i]:
                cost[i] = c
                prev[i] = j
    bounds = []
    i = NS
    while i > 0:
        j = prev[i]
        bounds.append((j, i - j, max(1, int(capr[i - 1]))))
        i = j
    bounds.reverse()
    groups = []
    col_off = 0
    ck_off = 0
    for s0, n, C in bounds:
        cols = PITCH * C * n
        nb = (cols + BANK - 1) // BANK
        groups.append(dict(s0=s0, n=n, C=C, off=col_off, nb=nb,
                           spc=max(1, 128 // C), ckoff=ck_off))
        col_off += nb * BANK
        ck_off += 3 * n
    return groups, col_off // BANK, ck_off


def _groups_key(groups):
    return tuple((g["s0"], g["n"], g["C"]) for g in groups)


def _plan(trajectory, alpha, z, csg, colors):
    """Compute compaction plan + packed per-frame data for ALL frames."""
    T = trajectory.shape[0]
    od = np.argsort(z, kind="stable")[::-1]     # descending z = paint order
    traj = np.asarray(trajectory, np.float32)[:, 0, :]
    P = traj[:, : N * K * 2].reshape(T, N, K, 2)[:, od]
    alive = traj[:, N * K * 2:][:, od]
    v0 = P
    v1 = np.roll(P, -1, axis=2)
    e = v1 - v0
    area2 = np.sum(v0[..., 0] * v1[..., 1] - v1[..., 0] * v0[..., 1], axis=2)
    orient = np.sign(area2).astype(np.float32)[:, :, None]
    A = (-orient * e[..., 1] / SOFT).astype(np.float32)       # [T,N,K] gx coef
    B = (orient * e[..., 0] / SOFT).astype(np.float32)        # gy coef
    Cc = (orient * (e[..., 1] * v0[..., 0] - e[..., 0] * v0[..., 1]) / SOFT
          ).astype(np.float32)

    sig_alive = 1.0 / (1.0 + np.exp(-alive.astype(np.float32)))
    aeff = np.asarray(alpha, np.float32)[od][None, :] * sig_alive   # [T,N]
    aeff = np.clip(aeff, 1e-12, 1.0 - 1e-7)
    logit = np.log(aeff / (1.0 - aeff)).astype(np.float32)          # [T,N]
    ck = (np.asarray(colors, np.float32)[0][od]
          * (1.0 - np.asarray(csg)[od].astype(np.float32))[:, None])  # [N,3]

    ys = ((np.arange(H) + 0.5) / H).astype(np.float32)
    x0, x1 = 0.5 / W, (W - 0.5) / W

    # --- per-row exact feasible x-interval, then per-tile activity:
    # a prim is active in a tile iff some row of the tile has a feasible
    # x-interval intersecting the tile's x-range.
    cx0 = ((np.arange(NTX) * TX + 0.5) / W).astype(np.float32)
    cx1 = ((np.arange(NTX) * TX + TX - 0.5) / W).astype(np.float32)
    cnt = np.empty((T, NSLOT), np.int32)
    active = np.empty((T, N, NSLOT), bool)
    step = 32
    for t0 in range(0, T, step):
        sl = slice(t0, t0 + step)
        D = B[sl, :, :, None] * ys[None, None, None, :] + Cc[sl, :, :, None]
        Ae = A[sl, :, :, None]
        Asafe = np.where(Ae == 0, 1.0, Ae)
        lo = np.where(Ae > 0, (-MARGIN - D) / Asafe, x0)
        hi = np.where(Ae < 0, (-MARGIN - D) / Asafe, x1)
        lo = np.where((Ae == 0) & (D < -MARGIN), x1 + 1.0, lo)
        LO = np.maximum(x0, lo.max(axis=2))       # [t,N,H]
        HI = np.minimum(x1, hi.min(axis=2))
        LOr = LO.reshape(-1, N, NTY, TY)
        HIr = HI.reshape(-1, N, NTY, TY)
        act = (np.maximum(LOr[..., None], cx0[None, None, None, None, :])
               <= np.minimum(HIr[..., None], cx1[None, None, None, None, :])
               ).any(axis=3)                      # [t,N,NTY,NTX]
        act = act.reshape(-1, N, NSLOT)
        active[sl] = act
        cnt[sl] = act.sum(axis=1)

    # --- slots: tiles sorted ascending by count; adaptive groups
    order = np.argsort(cnt, axis=1, kind="stable")       # [T, NSLOT]
    scnt = np.take_along_axis(cnt, order, axis=1)
    capr = scnt.max(axis=0)                              # [NSLOT]
    groups, NBTOT, CKTOT = _make_groups(capr)

    # active prim indices per (t, tile), z-order preserved
    Cmax = max(g["C"] for g in groups)
    act_tr = np.transpose(active, (0, 2, 1))             # [T, S, N]
    idx = np.argsort(~act_tr, axis=2, kind="stable")[:, :, :Cmax]  # [T,S,Cmax]
    valid = np.take_along_axis(act_tr, idx, axis=2)      # [T,S,Cmax]

    Ah, Al = _split2(A)
    Bh, Bl = _split2(B)
    w6 = np.zeros((T, 6, NBTOT * BANK), fp16)
    ck2a = np.zeros((T, 128, max(CKTOT, 1)), fp16)
    tt = np.arange(T)[:, None, None]
    for g in groups:
        C, off, n_g, spc, ckoff = (g["C"], g["off"], g["n"],
                                   g["spc"], g["ckoff"])
        slots = order[:, g["s0"]:g["s0"] + n_g]           # [T, n_g] tile ids
        pidx = np.take_along_axis(idx, slots[:, :, None], axis=1)[:, :, :C]
        pval = np.take_along_axis(valid, slots[:, :, None], axis=1)[:, :, :C]
        Ah_g = Ah[tt, pidx].astype(np.float32)            # [T,n,C,K]
        Al_g = Al[tt, pidx].astype(np.float32)
        Bh_g = Bh[tt, pidx].astype(np.float32)
        Bl_g = Bl[tt, pidx].astype(np.float32)
        A_g = A[tt, pidx]
        B_g = B[tt, pidx]
        Cc_g = Cc[tt, pidx]
        tx0s = ((slots % NTX) * TX / W).astype(np.float32)[:, :, None, None]
        ty0s = ((slots // NTX) * TY / H).astype(np.float32)[:, :, None, None]
        Ct = (A_g * tx0s + B_g * ty0s + Cc_g).astype(np.float32)
        Cth, Ctl = _split2(Ct)
        lg = logit[tt, pidx].astype(np.float32)           # [T,n,C]
        lg = np.where(pval, lg, -30.0)
        lgh, lgl = _split2(lg)
        dead = ~pval[..., None]
        Ah_g = np.where(dead, 0.0, Ah_g)
        Al_g = np.where(dead, 0.0, Al_g)
        Bh_g = np.where(dead, 0.0, Bh_g)
        Bl_g = np.where(dead, 0.0, Bl_g)
        Cth = np.where(dead, fp16(0), Cth)
        Ctl = np.where(dead, fp16(0), Ctl)

        blk = np.zeros((T, n_g, C, PITCH, 6), fp16)
        blk[..., 0, 4] = lgh                    # x col: Ct = logit
        blk[..., 0, 5] = lgl
        blk[..., 1:1 + K, 0] = Ah_g
        blk[..., 1:1 + K, 1] = Al_g
        blk[..., 1:1 + K, 2] = Bh_g
        blk[..., 1:1 + K, 3] = Bl_g
        blk[..., 1:1 + K, 4] = Cth
        blk[..., 1:1 + K, 5] = Ctl
        flat = blk.reshape(T, n_g * C * PITCH, 6)
        w6[:, :, off:off + flat.shape[1]] = flat.transpose(0, 2, 1)

        ckg = ck[pidx] * pval[..., None]                  # [T,n,C,3]
        for s in range(n_g):
            part0 = (s % spc) * C
            ck2a[:, part0:part0 + C, ckoff + 3 * s:ckoff + 3 * s + 3] = \
                ckg[:, s]

    # static lhsT: within-tile pixel offsets (exact in fp16)
    p = np.arange(128)
    ox = ((p % TX + 0.5) / W).astype(fp16)
    oy = ((p // TX + 0.5) / H).astype(fp16)
    G6 = np.zeros((6, 128), fp16)
    G6[0] = ox
    G6[1] = ox
    G6[2] = oy
    G6[3] = oy
    G6[4] = 1.0
    G6[5] = 1.0
    ident = np.eye(128, dtype=fp16)

    return dict(w6=w6, ck2a=ck2a, G6=G6, ident=ident, groups=groups,
                nbtot=NBTOT, cktot=max(CKTOT, 1), order=order)


def _host_prep(trajectory, colors, alpha, z, csg):
    plan = _plan(trajectory, alpha, z, csg, colors)
    in_maps = []
    for c in range(N_CORES):
        fr = slice(c * F, (c + 1) * F)
        in_maps.append({
            "g6": np.ascontiguousarray(plan["G6"]),
            "ident": np.ascontiguousarray(plan["ident"]),
            "w6": np.ascontiguousarray(plan["w6"][fr]),
            "ck2a": np.ascontiguousarray(plan["ck2a"][fr]),
        })
    return in_maps, plan


# ---------------------------------------------------------------------------
# device program
# ---------------------------------------------------------------------------

def _build_nc(n_frames, groups, nbtot, cktot):
    import concourse.bass as bass
    import concourse.bacc as bacc
    import concourse.tile as tile
    from concourse import mybir
    from contextlib import ExitStack

    dt = mybir.dt
    AF = mybir.ActivationFunctionType
    ALU = mybir.AluOpType
    AX = mybir.AxisListType

    nc = bacc.Bacc(None)
    g6_d = nc.dram_tensor("g6", [6, 128], dt.float16, kind="ExternalInput")
    ident_d = nc.dram_tensor("ident", [128, 128], dt.float16,
                             kind="ExternalInput")
    w6_d = nc.dram_tensor("w6", [n_frames, 6, nbtot * BANK], dt.float16,
                          kind="ExternalInput")
    ck2a_d = nc.dram_tensor("ck2a", [n_frames, 128, cktot], dt.float16,
                            kind="ExternalInput")
    out_d = nc.dram_tensor("out", [n_frames, NSLOT, 128, 3], dt.float32,
                           kind="ExternalOutput")

    with tile.TileContext(nc) as tc:
        with ExitStack() as ctx:
            singles = ctx.enter_context(tc.tile_pool(name="singles", bufs=1))
            w6_pool = ctx.enter_context(tc.tile_pool(name="w6", bufs=3))
            ck_pool = ctx.enter_context(tc.tile_pool(name="ck", bufs=2))
            sp_pool = ctx.enter_context(tc.tile_pool(name="sp", bufs=3))
            a_pool = ctx.enter_context(tc.tile_pool(name="a", bufs=3))
            om_pool = ctx.enter_context(tc.tile_pool(name="om", bufs=3))
            tt_pool = ctx.enter_context(tc.tile_pool(name="tt", bufs=3))
            w_pool = ctx.enter_context(tc.tile_pool(name="w", bufs=3))
            wt_pool = ctx.enter_context(tc.tile_pool(name="wt", bufs=3))
            fb_pool = ctx.enter_context(tc.tile_pool(name="fb", bufs=2))
            s_psum = ctx.enter_context(
                tc.tile_pool(name="s_ps", bufs=2, space="PSUM"))
            t_psum = ctx.enter_context(
                tc.tile_pool(name="t_ps", bufs=2, space="PSUM"))
            c_psum = ctx.enter_context(
                tc.tile_pool(name="c_ps", bufs=2, space="PSUM"))

            g6_sb = singles.tile([6, 128], dt.float16)
            nc.sync.dma_start(out=g6_sb, in_=g6_d[:])
            ones16 = singles.tile([128, 1], dt.float16)
            nc.vector.memset(ones16, 1.0)
            ident_sb = singles.tile([128, 128], dt.float16)
            nc.sync.dma_start(out=ident_sb, in_=ident_d[:])
            d1b = {}
            for gi, g in enumerate(groups):
                C, n_g = g["C"], g["n"]
                t2 = singles.tile([128, n_g * (C + 1)], dt.float16,
                                  tag=f"d1b{gi}")
                nc.vector.memset(t2, 0.0)
                r2 = bass.AP(tensor=t2.tensor, offset=t2.offset,
                             ap=[t2.ap[0], [C + 1, n_g], [1, 1]])
                nc.vector.memset(r2, 1.0)
                d1b[gi] = t2

            for t in range(n_frames):
                ck_sb = ck_pool.tile([128, cktot], dt.float16, tag="ck")
                nc.sync.dma_start(out=ck_sb, in_=ck2a_d[t])
                co_ps = c_psum.tile([128, BANK], dt.float32, tag="co")
                # front half: stream matmuls + sigmoid + segmented product.
                # Emitted for ALL groups before any back-half so the PE
                # queue never stalls on a transpose waiting for the DVE
                # chain of an earlier group.
                a_sbs = {}
                for gi, g in enumerate(groups):
                    C, off, nb, n_g = g["C"], g["off"], g["nb"], g["n"]
                    gcols = nb * BANK
                    w6_sb = w6_pool.tile([6, gcols], dt.float16, tag="w6")
                    nc.gpsimd.dma_start(out=w6_sb,
                                        in_=w6_d[t, :, off:off + gcols])
                    sp_sb = sp_pool.tile([128, gcols], dt.float16, tag="sp")
                    used = PITCH * C * n_g
                    for b0 in range(0, nb, 2):
                        nbk = min(2, nb - b0)
                        s_ps = s_psum.tile([128, 2 * BANK], dt.float32, tag="s")
                        for b in range(nbk):
                            nc.tensor.matmul(
                                s_ps[:, b * BANK:(b + 1) * BANK],
                                lhsT=g6_sb,
                                rhs=w6_sb[:, (b0 + b) * BANK:(b0 + b + 1) * BANK],
                                start=True, stop=True)
                        ncols = min(nbk * BANK, used - b0 * BANK)
                        nc.scalar.activation(
                            sp_sb[:, b0 * BANK:b0 * BANK + ncols],
                            s_ps[:, :ncols], AF.Sigmoid)
                    # segmented product over pitch-13 segments
                    a_sb = a_pool.tile([128, C * n_g], dt.float16,
                                       tag=f"a{gi}")
                    sp_ap = bass.AP(tensor=sp_sb.tensor, offset=sp_sb.offset,
                                    ap=[sp_sb.ap[0], [PITCH, C * n_g],
                                        [1, PITCH]])
                    nc.vector.tensor_reduce(a_sb, sp_ap, AX.X, ALU.mult)
                    a_sbs[gi] = a_sb
                # back half: compositing + transpose + color matmuls
                for gi, g in enumerate(groups):
                    C, n_g = g["C"], g["n"]
                    spc, ckoff = g["spc"], g["ckoff"]
                    a_sb = a_sbs[gi]
                    om_sb = om_pool.tile([128, n_g * (C + 1)], dt.float16,
                                         tag="om")
                    r2 = bass.AP(tensor=om_sb.tensor, offset=om_sb.offset,
                                 ap=[om_sb.ap[0], [C + 1, n_g], [1, 1]])
                    nc.gpsimd.memset(r2, 1.0)
                    om_ap = bass.AP(tensor=om_sb.tensor,
                                    offset=om_sb.offset + 1,
                                    ap=[om_sb.ap[0], [C + 1, n_g], [1, C]])
                    nc.gpsimd.tensor_scalar(om_ap, a_sb, -1.0, 1.0,
                                            ALU.mult, ALU.add)
                    tt_sb = tt_pool.tile([128, n_g * (C + 1)], dt.float16,
                                         tag="tt")
                    nc.vector.tensor_tensor_scan(
                        out=tt_sb, data0=om_sb, data1=d1b[gi],
                        initial=ones16[:, 0:1], op0=ALU.mult, op1=ALU.max)
                    w_sb = w_pool.tile([128, n_g * C], dt.float16, tag="w")
                    t0_ap = bass.AP(tensor=tt_sb.tensor, offset=tt_sb.offset,
                                    ap=[tt_sb.ap[0], [C + 1, n_g], [1, C]])
                    t1_ap = bass.AP(tensor=tt_sb.tensor,
                                    offset=tt_sb.offset + 1,
                                    ap=[tt_sb.ap[0], [C + 1, n_g], [1, C]])
                    nc.gpsimd.tensor_tensor(w_sb, t0_ap, t1_ap, ALU.subtract)
                    nchunk = (n_g + spc - 1) // spc
                    for j in range(nchunk):
                        ns = min(spc, n_g - j * spc)
                        ccols = ns * C
                        wt_ps = t_psum.tile([128, 1024], dt.float16, tag="wt")
                        nc.tensor.transpose(
                            wt_ps[:ccols, :128],
                            w_sb[:, j * spc * C:j * spc * C + ccols],
                            ident_sb)
                        wt_sb = wt_pool.tile([128, 128], dt.float16, tag="wts")
                        nc.vector.tensor_copy(wt_sb[:ccols, :],
                                              wt_ps[:ccols, :128])
                        s0 = j * spc
                        nc.tensor.matmul(
                            co_ps[:, (g["s0"] + s0) * 3:
                                  (g["s0"] + s0 + ns) * 3],
                            lhsT=wt_sb[0:ccols, :],
                            rhs=ck_sb[0:ccols,
                                      ckoff + 3 * s0:ckoff + 3 * (s0 + ns)],
                            start=True, stop=True)
                fb_sb = fb_pool.tile([128, NSLOT * 3], dt.float32, tag="fb")
                nc.scalar.copy(fb_sb, co_ps[:, :NSLOT * 3])
                src = fb_sb.rearrange("p (s ch) -> p s ch", ch=3)
                dst = out_d[t].rearrange("s p ch -> p s ch")
                nc.sync.dma_start(out=dst, in_=src)
    nc.finalize()
    return nc


def _get_program(n_frames, groups, nbtot, cktot):
    key = (n_frames, _groups_key(groups), nbtot, cktot)
    if key not in _CACHE:
        _CACHE[key] = _build_nc(n_frames, groups, nbtot, cktot)
    return _CACHE[key]


def _enable_jax_cache():
    try:
        import jax
        if jax.config.jax_compilation_cache_dir is None:
            jax.config.update("jax_compilation_cache_dir", "/tmp/jax_bass_cache")
            jax.config.update("jax_persistent_cache_min_entry_size_bytes", -1)
            jax.config.update("jax_persistent_cache_min_compile_time_secs", 0.5)
    except Exception:
        pass


def _unpermute(dev, order):
    """dev [T, slot, 128, 3] -> video [T, H, W, 3] (tile un-permute)."""
    T = dev.shape[0]
    video = np.empty((T, NSLOT, TY, TX, 3), np.float32)
    tt = np.arange(T)[:, None]
    video[tt, order] = dev.reshape(T, NSLOT, TY, TX, 3)
    video = video.reshape(T, NTY, NTX, TY, TX, 3)
    video = video.transpose(0, 1, 3, 2, 4, 5).reshape(T, H, W, 3)
    return video


def kernel(trajectory, colors, alpha, z, csg):
    from concourse.bass_utils import run_bass_kernel_spmd

    _enable_jax_cache()

    in_maps, plan = _host_prep(
        np.asarray(trajectory), np.asarray(colors), np.asarray(alpha),
        np.asarray(z), np.asarray(csg))
    nc = _get_program(F, plan["groups"], plan["nbtot"], plan["cktot"])
    res = run_bass_kernel_spmd(nc, in_maps, core_ids=list(range(N_CORES)))
    outs = [res.results[c]["out"] for c in range(N_CORES)]
    dev = np.concatenate(outs, axis=0)          # [192, slot, pix, 3]
    video = _unpermute(dev, plan["order"])
    return video[None].astype(np.float32)


if __name__ == "__main__":
    import time
    d = np.load("/root/problem/ref_cache.npz")
    t0 = time.time()
    in_maps, plan = _host_prep(d["trajectory"], d["colors"], d["alpha"],
                               d["z"], d["csg"])
    print(f"host prep: {time.time()-t0:.1f}s nbtot={plan['nbtot']}")
    print("groups:", _groups_key(plan["groups"]))
    t0 = time.time()
    nc = _build_nc(2, plan["groups"], plan["nbtot"], plan["cktot"])
    print(f"build 2f: {time.time()-t0:.1f}s")
